# revision 30
# baseline (speedup 1.0000x reference)
"""Causal self-attention (B=4, T=2048, C=1024, H=16) on 8 trn2 NeuronCores.

Sharding: tensor-parallel over heads. Each core owns 2 heads:
  - Wqkv columns for its heads ([1024, 3*128], q-part pre-scaled by 1/sqrt(d))
  - Wproj rows for its heads ([128, 1024])
  - full x (transposed to [C, B*T] on host so the contraction dim lands on
    SBUF partitions)
Each core computes its partial output projection [C, B*T] in bf16; the host
sums the 8 partials (the row-sharded Wproj reduction) and un-transposes.

Perf notes (measured on HW):
  - All matmul operands bf16: fp32r streams at ~2 cycles/row, bf16 at 1.
  - Concurrent ACT reads of PSUM stall in-flight matmuls ~2x; DVE
    tensor_copy/tensor_scalar_add with bf16 output do not. So phase C is
    evacuated by DVE; exp (only ACT can) stays on ACT and its stall is the
    main remaining tax.
  - Softmax 1/l: l is accumulated on PSUM partition 0 via a ones-column in
    the V operand (lhsT free index 0), reciprocal on DVE, partition 0
    broadcast by GPSIMD (partition_broadcast reads physical partition 0 on
    HW -- the AP partition offset is ignored, hence the layout), multiply
    on DVE. The PE is never involved in normalization.
  - V sits at partitions 64..127 of the PV output (DVE ops need 32-aligned
    partition windows; offset 64 allows 64 partitions).
  - x tile loads ride the GPSIMD DMA queue so batch b+1 loads are not
    head-of-line blocked behind batch b output stores on the sync queue.

On-core dataflow (per core, matmuls bf16 -> fp32 PSUM):
  A) QKV^T = Wslice^T @ x^T       -> ACT bias-add to QT/KT bf16, V staged
     fp32, PE-transposed to V natural [j, d] -> DVE cast into vaug bf16
  B) per (batch, 512-wide i-tile, head):
       S^T[j,i] = K Q^T            (128-row j-tiles, causal-skipped)
       P = exp(S^T) on ACT from PSUM -> bf16 (scores O(1), no max-sub)
       causal diagonal zeroed via gpsimd affine_select
       Y^T[d,i] += V_nat^T P^T     via matmul(lhsT=vaug, rhs=P)
       l[i]     += ones^T P        (vaug's ones column at index 0)
       1/l on DVE, GPSIMD partition-broadcast, DVE multiply, DMA into YT
  C) out^T[c,r] = Wproj_slice^T @ YT -> DVE bias-add cast to bf16,
     1024-wide stores
"""

import numpy as np
import ml_dtypes
from contextlib import ExitStack

import concourse.bacc as bacc
import concourse.bass as bass
import concourse.mybir as mybir
import concourse.tile as tile
from concourse.bass_utils import run_bass_kernel_spmd

NCORES = 8
C = 1024
H = 16
D = 64                 # head dim
HPC = H // NCORES      # heads per core = 2
FPC = HPC * D          # features per core = 128
KC = C // 128          # contraction chunks = 8
SCALE = 1.0 / 8.0      # 1/sqrt(D)

F32 = mybir.dt.float32
F32R = mybir.dt.float32r
BF16 = mybir.dt.bfloat16
AF = mybir.ActivationFunctionType
NPBF16 = ml_dtypes.bfloat16

_CACHE = {}
LAST_RESULT = None


def build_program(B, T):
    R = B * T
    TJ = T // 128          # 128-wide j (key) tiles per sequence
    TI = T // 512          # 512-wide i (query) tiles per sequence
    SB = HPC * TJ          # vaug stripes per batch
    assert T % 512 == 0

    nc = bacc.Bacc("TRN2", target_bir_lowering=False, debug=False,
                   num_devices=NCORES)
    xT = nc.dram_tensor("xT", [C, R], BF16, kind="ExternalInput").ap()
    wqkv = nc.dram_tensor("wqkv", [C, 3 * FPC], BF16, kind="ExternalInput").ap()
    bqkv = nc.dram_tensor("bqkv", [3 * FPC], F32, kind="ExternalInput").ap()
    wp = nc.dram_tensor("wp", [FPC, C], BF16, kind="ExternalInput").ap()
    bp = nc.dram_tensor("bp", [C], F32, kind="ExternalInput").ap()
    ident = nc.dram_tensor("ident", [128, D], F32, kind="ExternalInput").ap()
    ones64 = nc.dram_tensor("ones64", [128, 64], BF16, kind="ExternalInput").ap()
    vones = nc.dram_tensor("vones", [128, B * SB], BF16,
                           kind="ExternalInput").ap()
    outT = nc.dram_tensor("outT", [C, R], BF16, kind="ExternalOutput").ap()

    with tile.TileContext(nc) as tc, ExitStack() as ctx:
        const = ctx.enter_context(tc.tile_pool(name="const", bufs=1))
        big = ctx.enter_context(tc.tile_pool(name="big", bufs=1))
        xpool = ctx.enter_context(tc.tile_pool(name="xpool", bufs=6))
        vspool = ctx.enter_context(tc.tile_pool(name="vspool", bufs=2))
        ptpool = ctx.enter_context(tc.tile_pool(name="ptpool", bufs=4))
        bcpool = ctx.enter_context(tc.tile_pool(name="bcpool", bufs=2))
        recpool = ctx.enter_context(tc.tile_pool(name="recpool", bufs=2))
        opool = ctx.enter_context(tc.tile_pool(name="opool", bufs=3))
        ystpool = ctx.enter_context(tc.tile_pool(name="ystpool", bufs=2))
        psA = ctx.enter_context(tc.tile_pool(name="psA", bufs=3, space="PSUM"))
        psS = ctx.enter_context(tc.tile_pool(name="psS", bufs=3, space="PSUM"))
        psY = ctx.enter_context(tc.tile_pool(name="psY", bufs=2, space="PSUM"))

        # ---- constants ----
        w_sb = const.tile([128, KC, 3 * FPC], BF16)
        nc.sync.dma_start(out=w_sb,
                          in_=wqkv.rearrange("(kc p) c -> p kc c", p=128))
        wp_sb = const.tile([128, C], BF16)
        nc.sync.dma_start(out=wp_sb, in_=wp)
        bq_sb = const.tile([128, 3], F32)
        nc.sync.dma_start(out=bq_sb, in_=bqkv.rearrange("(cb p) -> p cb", p=128))
        bp_sb = const.tile([128, KC], F32)
        nc.sync.dma_start(out=bp_sb, in_=bp.rearrange("(cb p) -> p cb", p=128))
        # two stacked 64x64 identities, for PE-transposing per-head V^T slices
        identcol = const.tile([128, D], F32)
        nc.sync.dma_start(out=identcol, in_=ident)
        ones_sb = const.tile([128, 64], BF16)
        nc.sync.dma_start(out=ones_sb, in_=ones64)

        # per-batch buffers so attention on batch b overlaps QKV of batch b+1
        qts, kts, yts, vaugs = [], [], [], []
        for b in range(B):
            qts.append(big.tile([128, T], BF16, name=f"qt{b}", tag=f"qt{b}"))
            kts.append(big.tile([128, T], BF16, name=f"kt{b}", tag=f"kt{b}"))
            yts.append(big.tile([128, T], BF16, name=f"yt{b}", tag=f"yt{b}"))
            v = big.tile([128, 64 + D, SB], BF16, name=f"va{b}",
                         tag=f"va{b}")
            nc.sync.dma_start(out=v[:, 0, :],
                              in_=vones[:, b * SB:(b + 1) * SB])
            nc.vector.memset(v[:, 1:64, :], 0.0)
            vaugs.append(v)

        for b in range(B):
            qts.append(big.tile([128, T], BF16, name=f"qt{b}", tag=f"qt{b}"))
            kts.append(big.tile([128, T], BF16, name=f"kt{b}", tag=f"kt{b}"))
            yts.append(big.tile([128, T], BF16, name=f"yt{b}", tag=f"yt{b}"))
            v = big.tile([128, 64 + D, SB], BF16, name=f"va{b}",
                         tag=f"va{b}")
            nc.sync.dma_start(out=v[:, 0, :],
                              in_=vones[:, b * SB:(b + 1) * SB])
            nc.vector.memset(v[:, 1:64, :], 0.0)
            vaugs.append(v)

        def emit_proj_chunk(b, lt):
            # output projection for one 512-wide token window (phase C,
            # interleaved into phase B one i-tile behind as PE filler)
            yt = yts[b]
            for ct in range(KC):
                ps_o = psS.tile([128, 512], F32, tag="s", name="ps_o")
                nc.tensor.matmul(
                    ps_o[:, :],
                    lhsT=wp_sb[:, ct * 128:(ct + 1) * 128],
                    rhs=yt[:, lt * 512:(lt + 1) * 512],
                    start=True, stop=True,
                )
                ost = opool.tile([128, 512], BF16, tag="o", name="ost")
                nc.vector.tensor_scalar_add(ost[:, :], ps_o[:, :],
                                            bp_sb[:, ct:ct + 1])
                nc.sync.dma_start(
                    out=outT[ct * 128:(ct + 1) * 128,
                             b * T + lt * 512:b * T + (lt + 1) * 512],
                    in_=ost[:, :],
                )

        for b in range(B):
            qt, kt, yt, vaug = qts[b], kts[b], yts[b], vaugs[b]
            # ---- phase A(b): QKV projection + V transpose ----
            for lt in range(T // 512):
                l0 = lt * 512
                r0 = b * T + l0
                ps_q = psA.tile([128, 512], F32, tag="a")
                ps_k = psA.tile([128, 512], F32, tag="a")
                ps_v = psA.tile([128, 512], F32, tag="a")
                pss = [ps_q, ps_k, ps_v]
                for k in range(KC):
                    xt = xpool.tile([128, 512], BF16, tag="xt")
                    nc.gpsimd.dma_start(
                        out=xt, in_=xT[k * 128:(k + 1) * 128, r0:r0 + 512])
                    for ci in range(3):
                        nc.tensor.matmul(
                            pss[ci][:, :],
                            lhsT=w_sb[:, k, ci * FPC:(ci + 1) * FPC],
                            rhs=xt[:, :],
                            start=(k == 0), stop=(k == KC - 1),
                        )
                nc.scalar.activation(qt[:, l0:l0 + 512], ps_q[:, :],
                                     AF.Identity, bias=bq_sb[:, 0:1])
                nc.scalar.activation(kt[:, l0:l0 + 512], ps_k[:, :],
                                     AF.Identity, bias=bq_sb[:, 1:2])
                vstage = vspool.tile([128, 512], F32, tag="vs")
                nc.scalar.activation(vstage[:, :], ps_v[:, :], AF.Identity,
                                     bias=bq_sb[:, 2:3])
                for h in range(HPC):
                    for jb in range(4):
                        ps_t = psS.tile([128, 512], F32, tag="s")
                        nc.tensor.transpose(
                            ps_t[:, 0:D],
                            vstage[h * 64:(h + 1) * 64,
                                   jb * 128:(jb + 1) * 128],
                            identcol[h * 64:(h + 1) * 64, :],
                        )
                        s = h * TJ + lt * 4 + jb
                        nc.vector.tensor_copy(vaug[:, 64:64 + D, s],
                                              ps_t[:, 0:D])

            # ---- phase B(b): attention (heads interleaved per i-tile so
            # one head's reciprocal hides behind the other's matmuls) ----
            for it in range(TI):
                i0 = it * 512
                njt = (i0 + 512) // 128
                ps_ys = []
                for h in range(HPC):
                    h0 = h * 64
                    ps_y = psY.tile([128, 512], F32, tag="y")
                    ps_ys.append(ps_y)
                    for jj in range(njt):
                        j0 = jj * 128
                        off = max(0, j0 - i0)
                        w = 512 - off
                        ps_s = psS.tile([128, 512], F32, tag="s")
                        nc.tensor.matmul(
                            ps_s[:, 0:w],
                            lhsT=kt[h0:h0 + 64, j0:j0 + 128],
                            rhs=qt[h0:h0 + 64, i0 + off:i0 + 512],
                            start=True, stop=True,
                        )
                        pt = ptpool.tile([128, 512], BF16, tag="pt")
                        nc.scalar.activation(pt[:, 0:w], ps_s[:, 0:w], AF.Exp)
                        if j0 >= i0:
                            # diagonal block: zero P where j > i
                            nc.gpsimd.affine_select(
                                out=pt[:, 0:128], in_=pt[:, 0:128],
                                compare_op=mybir.AluOpType.is_ge,
                                fill=0.0, base=0,
                                pattern=[[1, 128]], channel_multiplier=-1,
                            )
                        nc.tensor.matmul(
                            ps_y[:, off:512],
                            lhsT=vaug[:, :, h * TJ + jj],
                            rhs=pt[:, 0:w],
                            start=(jj == 0), stop=(jj == njt - 1),
                        )
                    # 1/l on DVE (l is on partition 0), partition-broadcast
                    # on GPSIMD, multiply on DVE -- the PE is not involved
                    # in normalization at all
                    rec = recpool.tile([128, 512], F32, tag="rec",
                                       name="rec", bufs=2)
                    nc.vector.reciprocal(rec[0:1, :], ps_y[0:1, :])
                    ps_ys[h] = (ps_y, rec)
                for h in range(HPC):
                    ps_y, rec = ps_ys[h]
                    bc = bcpool.tile([128, 512], F32, tag="bc")
                    nc.gpsimd.partition_broadcast(bc[0:128, :], rec[0:1, :],
                                                  channels=128)
                    yst = ystpool.tile([128, 512], BF16, tag="yst")
                    nc.vector.tensor_mul(yst[64:128, :], ps_y[64:128, :],
                                         bc[64:128, :])
                    nc.sync.dma_start(
                        out=yt[h * 64:(h + 1) * 64, i0:i0 + 512],
                        in_=yst[64:128, :])

            # ---- phase C(b): output projection (host sums partials) ----
            for ct in range(KC):
                for lt2 in range(T // 1024):
                    ost = opool.tile([128, 1024], BF16, tag="o", name="ost")
                    for half in range(2):
                        lt = lt2 * 2 + half
                        ps_o = psS.tile([128, 512], F32, tag="s", name="ps_o")
                        nc.tensor.matmul(
                            ps_o[:, :],
                            lhsT=wp_sb[:, ct * 128:(ct + 1) * 128],
                            rhs=yt[:, lt * 512:(lt + 1) * 512],
                            start=True, stop=True,
                        )
                        nc.vector.tensor_scalar_add(
                            ost[:, half * 512:(half + 1) * 512], ps_o[:, :],
                            bp_sb[:, ct:ct + 1])
                    nc.sync.dma_start(
                        out=outT[ct * 128:(ct + 1) * 128,
                                 b * T + lt2 * 1024:b * T + (lt2 + 1) * 1024],
                        in_=ost[:, :],
                    )


    nc.compile()
    return nc


def make_in_maps(x, Wqkv, bqkv, Wproj, bproj):
    Bx, Tx, Cx = x.shape
    R = Bx * Tx
    xTh = np.ascontiguousarray(
        x.reshape(R, Cx).T.astype(np.float32)).astype(NPBF16)
    eye = np.eye(D, dtype=np.float32)
    ident_h = np.ascontiguousarray(np.concatenate([eye, eye], axis=0))
    S = Bx * HPC * (Tx // 128)
    vones_h = np.ones((128, S), NPBF16)
    ones64_h = np.ones((128, 64), NPBF16)
    in_maps = []
    for i in range(NCORES):
        cs = slice(i * FPC, (i + 1) * FPC)
        wq = Wqkv[:, 0 * C:1 * C][:, cs] * SCALE
        wk = Wqkv[:, 1 * C:2 * C][:, cs]
        wv = Wqkv[:, 2 * C:3 * C][:, cs]
        wqkv_s = np.ascontiguousarray(
            np.concatenate([wq, wk, wv], axis=1).astype(np.float32)
        ).astype(NPBF16)
        bq = bqkv[0 * C:1 * C][cs] * SCALE
        bk = bqkv[1 * C:2 * C][cs]
        bv = bqkv[2 * C:3 * C][cs]
        bqkv_s = np.ascontiguousarray(
            np.concatenate([bq, bk, bv]).astype(np.float32))
        wp_s = np.ascontiguousarray(
            Wproj[cs, :].astype(np.float32)).astype(NPBF16)
        bp_s = (bproj.astype(np.float32) if i == 0
                else np.zeros((C,), np.float32))
        in_maps.append({
            "xT": xTh,
            "wqkv": wqkv_s,
            "bqkv": bqkv_s,
            "wp": wp_s,
            "bp": np.ascontiguousarray(bp_s),
            "ident": ident_h,
            "vones": vones_h,
            "ones64": ones64_h,
        })
    return in_maps


def kernel(x, Wqkv, bqkv, Wproj, bproj, trace=False):
    global LAST_RESULT
    x = np.asarray(x, dtype=np.float32)
    Wqkv = np.asarray(Wqkv, dtype=np.float32)
    bqkv = np.asarray(bqkv, dtype=np.float32)
    Wproj = np.asarray(Wproj, dtype=np.float32)
    bproj = np.asarray(bproj, dtype=np.float32)
    Bx, Tx, Cx = x.shape
    assert Cx == C

    key = (Bx, Tx)
    if key not in _CACHE:
        _CACHE[key] = build_program(Bx, Tx)
    nc = _CACHE[key]

    in_maps = make_in_maps(x, Wqkv, bqkv, Wproj, bproj)
    res = run_bass_kernel_spmd(nc, in_maps, list(range(NCORES)), trace=trace)
    LAST_RESULT = res
    acc = np.zeros((C, Bx * Tx), dtype=np.float32)
    for i in range(NCORES):
        acc += np.asarray(res.results[i]["outT"]).astype(np.float32)
    return np.ascontiguousarray(acc.T).reshape(Bx, Tx, Cx)
